# revision 10
# baseline (speedup 1.0000x reference)
"""Multi-head graph attention (GAT) Trainium2 kernel, 8-core SPMD.

Problem: h[4096,256], adj[4096,4096] bool, w[4,256,64], a_src/a_dst[4,64,1],
bias[64] -> out[4096,4,64]:
    h_prime = h @ w[k]                       per head
    s[i,j]  = src[i] + dst[j]                (rank-1!)
    scores  = leaky_relu(s, 0.2), masked by adj, softmax over j
    out     = attn @ h_prime + bias

Sharding: 8 cores = 2 head-groups x 4 row-blocks. Core c computes heads
[2*(c%2), 2*(c%2)+1] for output rows [1024*(c//2), 1024*(c//2)+1024).

Key algebra (per head, all on-device):
    exp(leaky(s)) = e^{0.2 s} * max(e^{0.8 s}, 1)
factors into row/col terms; the e^{0.2 src_i} column factor cancels in the
softmax.  With a = e^{0.8 src_i}, binv_j = e^{-0.8 dst_j},
q_j = e^{dst_j - CB}:
    max(e^{0.8(src+dst)}, 1) = e^{0.8 dst_j} * max(a_i, binv_j)
so the unnormalized weight splits as
    P[j,i] = max(a_i, binv_j) * adj[j,i]          (ONE fused DVE op:
             scalar_tensor_tensor (a max binv) mult adjT)
    g[j,o] = q_j * hp[j,o],  g[j,64] = q_j        (the j-row factor rides
             the stationary matmul operand; col 64 gives the softmax
             denominator for free)
The epilogue transposes acc[65,512] -> [128,65] and divides by the
denominator column.  hp/dst are computed in 3-chunk psum strips so the
exp/copy ACT work batches into a handful of wide ops instead of ~200
narrow ones.
"""

import sys

sys.path.insert(0, "/opt/trn_rl_repo")

import numpy as np
import ml_dtypes

N = 4096          # nodes
F = 256           # f_in
O = 64            # f_out
NHEAD = 4
NCORES = 8
NH = 2            # heads per core
NI = 1024         # output rows per core
NCJ = N // 128    # 32 j-chunks
NSEG = NI // 512  # 2 segments of 512 in the i (free) dim
NSUB = NI // 128  # 8 i-subtiles of 128
CB = 20.0         # shift inside e^{dst - CB} to keep ranges safe
SCHUNK = 3        # j-chunks per hp/dst psum strip

_CACHE = {}


def _build():
    import concourse.bass as bass
    import concourse.bacc as bacc
    import concourse.mybir as mybir
    import concourse.tile as tile
    from concourse.bass import ts

    from concourse.masks import make_identity

    f32 = mybir.dt.float32
    bf16 = mybir.dt.bfloat16
    Alu = mybir.AluOpType
    Act = mybir.ActivationFunctionType

    nc = bacc.Bacc()
    hT_d = nc.declare_dram_parameter("hT", [F, N], bf16, isOutput=False)
    hTi_d = nc.declare_dram_parameter("hTi", [F, NI], bf16, isOutput=False)
    adjT_d = nc.declare_dram_parameter("adjT", [8 * 128, 4 * NI], bf16, isOutput=False)
    wr_d = nc.declare_dram_parameter("wr", [F, NH * O], bf16, isOutput=False)
    wTr_d = nc.declare_dram_parameter("wTr", [O, NH * F], bf16, isOutput=False)
    avec_d = nc.declare_dram_parameter("avec", [O, 2 * NH], bf16, isOutput=False)
    out_d = nc.declare_dram_parameter("out", [NH, 128, NSUB * O], f32, isOutput=True)

    # strips of SCHUNK j-chunks; each strip is one [128, n*130] psum region
    strips = []
    c0 = 0
    while c0 < NCJ:
        n = min(SCHUNK, NCJ - c0)
        strips.append((c0, n))
        c0 += n

    with tile.TileContext(nc) as tc:
        with (
            tc.tile_pool(name="sb", bufs=1) as sb,
            tc.tile_pool(name="sbr", bufs=2) as sbr,
            tc.tile_pool(name="sbo", bufs=3) as sbo,
            tc.tile_pool(name="psc", bufs=3, space="PSUM") as psc,
            tc.tile_pool(name="pacc", bufs=1, space="PSUM") as pacc,
        ):
            # ---- static SBUF tensors ----
            hT_sb = sb.tile([128, 2, N], bf16, name="hT_sb")
            hTi_sb = sb.tile([128, 2, NI], bf16, name="hTi_sb")
            adjT_sb = sb.tile([128, NCJ, NI], bf16, name="adjT_sb")
            wTr_sb = sb.tile([O, NH, F], bf16, name="wTr_sb")
            avec_sb = sb.tile([O, 2 * NH], bf16, name="avec_sb")
            wall_sb = sb.tile([128, 2, NH * O + NH], bf16, name="wall_sb")
            vsrc_sb = sb.tile([128, 2, NH], bf16, name="vsrc_sb")
            ones_row = sb.tile([1, 128], bf16, name="ones_row")
            esrc_row = sb.tile([1, NH, NI], bf16, name="esrc_row")
            esrc_rep = sb.tile([128, NH, NI], bf16, name="esrc_rep")
            g_sb = sb.tile([128, NH, NCJ, O + 1], bf16, name="g_sb")
            binv_sb = sb.tile([128, NCJ, NH], bf16, name="binv_sb")
            q_sb = sb.tile([128, NCJ, NH], f32, name="q_sb")
            ostage = sb.tile([128, NH, NSUB, O], f32, name="ostage")
            negcb = sb.tile([128, 1], f32, name="negcb")
            nc.vector.memset(negcb[:, :], -CB)
            zerob = sb.tile([128, 1], f32, name="zerob")
            nc.vector.memset(zerob[:, :], 0.0)
            ident = sb.tile([128, 128], f32, name="ident")
            make_identity(nc, ident[:, :])

            # ---- DMA in ----  (small control tensors FIRST so the prologue
            # matmul chain can start while the bulk hT/adjT loads stream in)
            nc.sync.dma_start(
                hTi_sb, hTi_d[:, :].rearrange("(fc p) i -> p fc i", p=128)
            )
            wTr_r = wTr_d[:, :].rearrange("o (h f) -> o h f", h=NH)
            for h in range(NH):
                nc.sync.dma_start(wTr_sb[:, h, :], wTr_r[:, h, :])
            nc.sync.dma_start(avec_sb, avec_d[:, :])
            nc.sync.dma_start(
                wall_sb[:, :, 0 : NH * O],
                wr_d[:, :].rearrange("(fc p) m -> p fc m", p=128),
            )
            # adjT is host-pre-tiled as [8 groups, 128 partitions, 4*NI]
            adjT_r = adjT_d[:, :].rearrange("(g p) x -> g p x", p=128)
            nc.sync.dma_start(
                adjT_sb[:, 0:4, :].rearrange("p c i -> p (c i)"), adjT_r[0]
            )
            hT_r = hT_d[:, :].rearrange("(fc p) j -> p fc j", p=128)
            nc.sync.dma_start(hT_sb[:, :, 0:512], hT_r[:, :, 0:512])
            nc.sync.dma_start(hT_sb[:, :, 512:N], hT_r[:, :, 512:N])
            for g in range(1, 7, 2):
                nc.sync.dma_start(
                    adjT_sb[:, 4 * g : 4 * g + 8, :].rearrange(
                        "p (g c) i -> p g (c i)", g=2
                    ),
                    adjT_r[g : g + 2].rearrange("g p x -> p g x"),
                )
            nc.sync.dma_start(
                adjT_sb[:, 28:32, :].rearrange("p c i -> p (c i)"), adjT_r[7]
            )

            nc.vector.memset(ones_row[:, :], 1.0)

            # ---- v vectors: v[f] = sum_o wT[o,f] * a[o]  (src, dst per head)
            v_ps = psc.tile([128, 512], f32, name="v_ps", tag="scr")
            for h in range(NH):
                for fc in range(2):
                    nc.tensor.matmul(
                        v_ps[:, 4 * fc + 2 * h : 4 * fc + 2 * h + 2],
                        lhsT=wTr_sb[:, h, ts(fc, 128)],
                        rhs=avec_sb[:, 2 * h : 2 * h + 2],
                        start=True,
                        stop=True,
                    )
            for fc in range(2):
                for h in range(NH):
                    nc.scalar.copy(
                        vsrc_sb[:, fc, h : h + 1], v_ps[:, 4 * fc + 2 * h : 4 * fc + 2 * h + 1]
                    )
                    nc.scalar.copy(
                        wall_sb[:, fc, NH * O + h : NH * O + h + 1],
                        v_ps[:, 4 * fc + 2 * h + 1 : 4 * fc + 2 * h + 2],
                    )

            # ---- src rows for this core's i-block (both heads), then exp
            for h in range(NH):
                for seg in range(NSEG):
                    sr_ps = psc.tile([128, 512], f32, name=f"sr_ps_{h}_{seg}", tag="scr")
                    for fc in range(2):
                        nc.tensor.matmul(
                            sr_ps[0:1, :],
                            lhsT=vsrc_sb[:, fc, h : h + 1],
                            rhs=hTi_sb[:, fc, ts(seg, 512)],
                            start=(fc == 0),
                            stop=(fc == 1),
                        )
                    nc.scalar.activation(
                        esrc_row[0:1, h, ts(seg, 512)],
                        sr_ps[0:1, :],
                        Act.Exp,
                        scale=0.8,
                        bias=zerob[0:1, :],
                    )

            # ---- replicate e^{0.8 src} across partitions (K=1 ones matmul)
            for h in range(NH):
                for seg in range(NSEG):
                    rep_ps = psc.tile([128, 512], f32, name=f"rep_ps_{h}_{seg}", tag="scr")
                    nc.tensor.matmul(
                        rep_ps[:, :],
                        lhsT=ones_row[:, :],
                        rhs=esrc_row[0:1, h, ts(seg, 512)],
                        start=True,
                        stop=True,
                    )
                    nc.scalar.copy(esrc_rep[:, h, ts(seg, 512)], rep_ps[:, :])

            # ---- bmm accumulators: psum [65, 512] per (head, i-segment)
            acc = [
                pacc.tile([O + 1, 512], f32, name=f"acc{g}", tag=f"acc{g}")
                for g in range(NH * NSEG)
            ]

            # ---- strip producer: hp+dst matmuls, batched exps, g casts ----
            strip_tiles = {}

            def produce(si):
                if si >= len(strips):
                    return
                c0, n = strips[si]
                st = psc.tile([128, 512], f32, name=f"hp{si}", tag="scr")
                sv = st[:, 0 : n * 130].rearrange("p (c k) -> p c k", c=n)
                strip_tiles[si] = sv
                for cc in range(n):
                    for fc in range(2):
                        nc.tensor.matmul(
                            sv[:, cc, :],
                            lhsT=hT_sb[:, fc, ts(c0 + cc, 128)],
                            rhs=wall_sb[:, fc, :],
                            start=(fc == 0),
                            stop=(fc == 1),
                        )
                # binv = e^{-0.8 dst}  [128, n, NH]
                nc.scalar.activation(
                    binv_sb[:, c0 : c0 + n, :],
                    sv[:, :, NH * O : NH * O + NH],
                    Act.Exp,
                    scale=-0.8,
                    bias=zerob[:, :],
                )
                # q = e^{dst - CB}: f32 copy for cast scales + bf16 copy into
                # g's denominator column
                nc.scalar.activation(
                    q_sb[:, c0 : c0 + n, :],
                    sv[:, :, NH * O : NH * O + NH],
                    Act.Exp,
                    scale=1.0,
                    bias=negcb[:, :],
                )
                nc.scalar.activation(
                    g_sb[:, :, c0 : c0 + n, O : O + 1].rearrange(
                        "p h c x -> p h (c x)"
                    ),
                    sv[:, :, NH * O : NH * O + NH].rearrange("p c h -> p h c"),
                    Act.Exp,
                    scale=1.0,
                    bias=negcb[:, :],
                )
                # g[:, h, c, 0:O] = q * hp
                for cc in range(n):
                    for h in range(NH):
                        nc.scalar.activation(
                            g_sb[:, h, c0 + cc, 0:O],
                            sv[:, cc, ts(h, O)],
                            Act.Copy,
                            scale=q_sb[:, c0 + cc, h : h + 1],
                        )

            LOOKAHEAD = 2
            for si in range(LOOKAHEAD):
                produce(si)

            # ---- main loop: one fused DVE op + 2 accumulating matmuls per
            # (chunk, head)
            for si, (c0, n) in enumerate(strips):
                produce(si + LOOKAHEAD)
                for cc in range(n):
                    c = c0 + cc
                    for h in range(NH):
                        p_t = sbr.tile(
                            [128, NI], bf16, name=f"p_{h}_{c}", tag=f"P{h}", bufs=6
                        )
                        nc.vector.scalar_tensor_tensor(
                            out=p_t[:, :],
                            in0=esrc_rep[:, h, :],
                            scalar=binv_sb[:, c, h : h + 1],
                            in1=adjT_sb[:, c, :],
                            op0=Alu.max,
                            op1=Alu.mult,
                        )
                        for seg in range(NSEG):
                            nc.tensor.matmul(
                                acc[h * NSEG + seg][:, :],
                                lhsT=g_sb[:, h, c, :],
                                rhs=p_t[:, ts(seg, 512)],
                                start=(c == 0),
                                stop=(c == NCJ - 1),
                            )

            # ---- epilogue: transpose [65,512] -> 4x[128,65], divide by the
            # denominator column, stage, DMA out per head
            for h in range(NH):
                for seg in range(NSEG):
                    a_ps = acc[h * NSEG + seg]
                    tr_in = sbo.tile([O + 1, 512], f32, name=f"tr_{h}_{seg}", tag="trin")
                    nc.scalar.copy(tr_in[:, :], a_ps[:, :])
                    trp = psc.tile([128, 512], f32, name=f"trp_{h}_{seg}", tag="scr")
                    trv = trp[:, 0 : 4 * (O + 1)].rearrange("p (q k) -> p q k", q=4)
                    for q in range(4):
                        nc.tensor.transpose(
                            trv[:, q, :],
                            tr_in[:, ts(q, 128)],
                            ident[0 : O + 1, 0 : O + 1],
                        )
                    rec = sbr.tile([128, 4], f32, name=f"rec_{h}_{seg}", tag="rec")
                    nc.vector.reciprocal(
                        rec[:, :], trv[:, :, O : O + 1].rearrange("p q x -> p (q x)")
                    )
                    for q in range(4):
                        nc.scalar.activation(
                            ostage[:, h, seg * 4 + q, :],
                            trv[:, q, 0:O],
                            Act.Copy,
                            scale=rec[:, q : q + 1],
                        )
                nc.sync.dma_start(
                    out_d[h, :, :], ostage[:, h, :, :].rearrange("p s o -> p (s o)")
                )

    nc.finalize()
    return nc


def _prep_inputs(h, adj, w, a_src, a_dst, bias):
    """Host-side sharding / layout prep (no reference math)."""
    h = np.asarray(h, dtype=np.float32)
    adj = np.asarray(adj)
    w = np.asarray(w, dtype=np.float32)
    a_src = np.asarray(a_src, dtype=np.float32)
    a_dst = np.asarray(a_dst, dtype=np.float32)
    bias = np.asarray(bias, dtype=np.float32)

    hT = np.ascontiguousarray(h.T)                       # [F, N]
    adjT = np.ascontiguousarray(adj.T).astype(ml_dtypes.bfloat16)  # [N, N] 0/1

    in_maps = []
    for c in range(NCORES):
        hb, ib = c % 2, c // 2
        heads = [2 * hb, 2 * hb + 1]
        i0 = NI * ib
        w2 = w[heads]                                    # [2, F, O]
        wr = np.ascontiguousarray(w2.transpose(1, 0, 2).reshape(F, NH * O))
        wTr = np.ascontiguousarray(
            np.concatenate([w2[0].T, w2[1].T], axis=1)   # [O, 2F]
        )
        avec = np.ascontiguousarray(
            np.stack(
                [a_src[heads[0], :, 0], a_dst[heads[0], :, 0],
                 a_src[heads[1], :, 0], a_dst[heads[1], :, 0]],
                axis=1,
            )
        )                                                # [O, 4]
        in_maps.append(
            {
                "hT": hT.astype(ml_dtypes.bfloat16),
                "hTi": np.ascontiguousarray(hT[:, i0 : i0 + NI]).astype(
                    ml_dtypes.bfloat16
                ),
                "adjT": np.ascontiguousarray(
                    adjT[:, i0 : i0 + NI]
                    .reshape(8, 4, 128, NI)
                    .transpose(0, 2, 1, 3)
                    .reshape(8 * 128, 4 * NI)
                ),
                "wr": wr.astype(ml_dtypes.bfloat16),
                "wTr": wTr.astype(ml_dtypes.bfloat16),
                "avec": avec.astype(ml_dtypes.bfloat16),
            }
        )
    return in_maps


def kernel(h, adj, w, a_src, a_dst, bias):
    from concourse.bass_utils import run_bass_kernel_spmd

    if "nc" not in _CACHE:
        _CACHE["nc"] = _build()
    nc = _CACHE["nc"]

    in_maps = _prep_inputs(h, adj, w, a_src, a_dst, bias)
    res = run_bass_kernel_spmd(nc, in_maps, list(range(NCORES))).results

    out = np.empty((N, NHEAD, O), dtype=np.float32)
    for c in range(NCORES):
        hb, ib = c % 2, c // 2
        arr = res[c]["out"]  # [NH, 128, NSUB*O]
        for hh in range(NH):
            blk = (
                arr[hh]
                .reshape(128, NSUB, O)
                .transpose(1, 0, 2)
                .reshape(NI, O)
            )
            out[NI * ib : NI * (ib + 1), 2 * hb + hh, :] = blk
    out += np.asarray(bias, dtype=np.float32).reshape(1, 1, O)
    return out


# revision 12
# speedup vs baseline: 1.1259x; 1.1259x over previous
"""Multi-head graph attention (GAT) Trainium2 kernel, 8-core SPMD.

Problem: h[4096,256], adj[4096,4096] bool, w[4,256,64], a_src/a_dst[4,64,1],
bias[64] -> out[4096,4,64]:
    h_prime = h @ w[k]                       per head
    s[i,j]  = src[i] + dst[j]                (rank-1!)
    scores  = leaky_relu(s, 0.2), masked by adj, softmax over j
    out     = attn @ h_prime + bias

Sharding: 8 cores = 2 head-groups x 4 row-blocks. Core c computes heads
[2*(c%2), 2*(c%2)+1] for output rows [1024*(c//2), 1024*(c//2)+1024).

Key algebra (per head, all on-device):
    exp(leaky(s)) = e^{0.2 s} * max(e^{0.8 s}, 1)
factors into row/col terms; the e^{0.2 src_i} column factor cancels in the
softmax.  With a = e^{0.8 src_i}, binv_j = e^{-0.8 dst_j},
q_j = e^{dst_j - CB}:
    max(e^{0.8(src+dst)}, 1) = e^{0.8 dst_j} * max(a_i, binv_j)
so the unnormalized weight splits as
    P[j,i] = max(a_i, binv_j) * adj[j,i]          (ONE fused DVE op:
             scalar_tensor_tensor (a max binv) mult adjT)
    g[j,o] = q_j * hp[j,o],  g[j,64] = q_j        (the j-row factor rides
             the stationary matmul operand; col 64 gives the softmax
             denominator for free)
The epilogue transposes acc[65,512] -> [128,65] and divides by the
denominator column.  hp/dst are computed in 3-chunk psum strips so the
exp/copy ACT work batches into a handful of wide ops instead of ~200
narrow ones.
"""

import sys

sys.path.insert(0, "/opt/trn_rl_repo")

import numpy as np
import ml_dtypes

N = 4096          # nodes
F = 256           # f_in
O = 64            # f_out
NHEAD = 4
NCORES = 8
NH = 2            # heads per core
NI = 1024         # output rows per core
NCJ = N // 128    # 32 j-chunks
NSEG = NI // 512  # 2 segments of 512 in the i (free) dim
NSUB = NI // 128  # 8 i-subtiles of 128
CB = 20.0         # shift inside e^{dst - CB} to keep ranges safe
SCHUNK = 3        # j-chunks per hp/dst psum strip

_CACHE = {}


def _build():
    import concourse.bass as bass
    import concourse.bacc as bacc
    import concourse.mybir as mybir
    import concourse.tile as tile
    from concourse.bass import ts

    from concourse.masks import make_identity

    f32 = mybir.dt.float32
    bf16 = mybir.dt.bfloat16
    Alu = mybir.AluOpType
    Act = mybir.ActivationFunctionType

    nc = bacc.Bacc()
    hT_d = nc.declare_dram_parameter("hT", [F, N], bf16, isOutput=False)
    hTi_d = nc.declare_dram_parameter("hTi", [F, NI], bf16, isOutput=False)
    adjT_d = nc.declare_dram_parameter("adjT", [8 * 128, 4 * NI], bf16, isOutput=False)
    wr_d = nc.declare_dram_parameter("wr", [F, NH * O], bf16, isOutput=False)
    wTr_d = nc.declare_dram_parameter("wTr", [O, NH * F], bf16, isOutput=False)
    avec_d = nc.declare_dram_parameter("avec", [O, 2 * NH], bf16, isOutput=False)
    out_d = nc.declare_dram_parameter("out", [NH, 128, NSUB * O], f32, isOutput=True)

    # strips of SCHUNK j-chunks; each strip is one [128, n*130] psum region
    strips = []
    c0 = 0
    while c0 < NCJ:
        n = min(SCHUNK, NCJ - c0)
        strips.append((c0, n))
        c0 += n

    with tile.TileContext(nc) as tc:
        with (
            tc.tile_pool(name="sb", bufs=1) as sb,
            tc.tile_pool(name="sbr", bufs=2) as sbr,
            tc.tile_pool(name="sbo", bufs=3) as sbo,
            tc.tile_pool(name="psc", bufs=3, space="PSUM") as psc,
            tc.tile_pool(name="pacc", bufs=1, space="PSUM") as pacc,
        ):
            # ---- static SBUF tensors ----
            hT_sb = sb.tile([128, 2, N], bf16, name="hT_sb")
            hTi_sb = sb.tile([128, 2, NI], bf16, name="hTi_sb")
            adjT_sb = sb.tile([128, NCJ, NI], bf16, name="adjT_sb")
            wTr_sb = sb.tile([O, NH, F], bf16, name="wTr_sb")
            avec_sb = sb.tile([O, 2 * NH], bf16, name="avec_sb")
            wall_sb = sb.tile([128, 2, NH * O + NH], bf16, name="wall_sb")
            vsrc_sb = sb.tile([128, 2, NH], bf16, name="vsrc_sb")
            ones_row = sb.tile([1, 128], bf16, name="ones_row")
            esrc_row = sb.tile([1, NH, NI], bf16, name="esrc_row")
            esrc_rep = sb.tile([128, NH, NI], bf16, name="esrc_rep")
            g_sb = sb.tile([128, NH, NCJ, O + 1], bf16, name="g_sb")
            binv_sb = sb.tile([128, NCJ, NH], f32, name="binv_sb")
            q_sb = sb.tile([128, NCJ, NH], f32, name="q_sb")
            ostage = sb.tile([128, NH, NSUB, O], f32, name="ostage")
            negcb = sb.tile([128, 1], f32, name="negcb")
            nc.vector.memset(negcb[:, :], -CB)
            zerob = sb.tile([128, 1], f32, name="zerob")
            nc.vector.memset(zerob[:, :], 0.0)
            ident = sb.tile([128, 128], f32, name="ident")
            make_identity(nc, ident[:, :])

            # ---- DMA in ----  (small control tensors FIRST so the prologue
            # matmul chain can start while the bulk hT/adjT loads stream in)
            nc.sync.dma_start(
                hTi_sb, hTi_d[:, :].rearrange("(fc p) i -> p fc i", p=128)
            )
            wTr_r = wTr_d[:, :].rearrange("o (h f) -> o h f", h=NH)
            for h in range(NH):
                nc.sync.dma_start(wTr_sb[:, h, :], wTr_r[:, h, :])
            nc.sync.dma_start(avec_sb, avec_d[:, :])
            nc.sync.dma_start(
                wall_sb[:, :, 0 : NH * O],
                wr_d[:, :].rearrange("(fc p) m -> p fc m", p=128),
            )
            # adjT is host-pre-tiled as [8 groups, 128 partitions, 4*NI]
            adjT_r = adjT_d[:, :].rearrange("(g p) x -> g p x", p=128)
            nc.sync.dma_start(
                adjT_sb[:, 0:4, :].rearrange("p c i -> p (c i)"), adjT_r[0]
            )
            hT_r = hT_d[:, :].rearrange("(fc p) j -> p fc j", p=128)
            nc.sync.dma_start(hT_sb[:, :, 0:512], hT_r[:, :, 0:512])
            nc.sync.dma_start(hT_sb[:, :, 512:N], hT_r[:, :, 512:N])
            for g in range(1, 7, 2):
                nc.sync.dma_start(
                    adjT_sb[:, 4 * g : 4 * g + 8, :].rearrange(
                        "p (g c) i -> p g (c i)", g=2
                    ),
                    adjT_r[g : g + 2].rearrange("g p x -> p g x"),
                )
            nc.sync.dma_start(
                adjT_sb[:, 28:32, :].rearrange("p c i -> p (c i)"), adjT_r[7]
            )

            nc.vector.memset(ones_row[:, :], 1.0)

            # ---- v vectors: v[f] = sum_o wT[o,f] * a[o]  (src, dst per head)
            v_ps = psc.tile([128, 512], f32, name="v_ps", tag="scr")
            for h in range(NH):
                for fc in range(2):
                    nc.tensor.matmul(
                        v_ps[:, 4 * fc + 2 * h : 4 * fc + 2 * h + 2],
                        lhsT=wTr_sb[:, h, ts(fc, 128)],
                        rhs=avec_sb[:, 2 * h : 2 * h + 2],
                        start=True,
                        stop=True,
                    )
            for fc in range(2):
                for h in range(NH):
                    nc.scalar.copy(
                        vsrc_sb[:, fc, h : h + 1], v_ps[:, 4 * fc + 2 * h : 4 * fc + 2 * h + 1]
                    )
                    nc.scalar.copy(
                        wall_sb[:, fc, NH * O + h : NH * O + h + 1],
                        v_ps[:, 4 * fc + 2 * h + 1 : 4 * fc + 2 * h + 2],
                    )

            # ---- src rows for this core's i-block (both heads), then exp
            for h in range(NH):
                for seg in range(NSEG):
                    sr_ps = psc.tile([128, 512], f32, name=f"sr_ps_{h}_{seg}", tag="scr")
                    for fc in range(2):
                        nc.tensor.matmul(
                            sr_ps[0:1, :],
                            lhsT=vsrc_sb[:, fc, h : h + 1],
                            rhs=hTi_sb[:, fc, ts(seg, 512)],
                            start=(fc == 0),
                            stop=(fc == 1),
                        )
                    nc.scalar.activation(
                        esrc_row[0:1, h, ts(seg, 512)],
                        sr_ps[0:1, :],
                        Act.Exp,
                        scale=0.8,
                        bias=zerob[0:1, :],
                    )

            # ---- replicate e^{0.8 src} across partitions (K=1 ones matmul)
            for h in range(NH):
                for seg in range(NSEG):
                    rep_ps = psc.tile([128, 512], f32, name=f"rep_ps_{h}_{seg}", tag="scr")
                    nc.tensor.matmul(
                        rep_ps[:, :],
                        lhsT=ones_row[:, :],
                        rhs=esrc_row[0:1, h, ts(seg, 512)],
                        start=True,
                        stop=True,
                    )
                    nc.scalar.copy(esrc_rep[:, h, ts(seg, 512)], rep_ps[:, :])

            # ---- bmm accumulators: psum [65, 512] per (head, i-segment)
            acc = [
                pacc.tile([O + 1, 512], f32, name=f"acc{g}", tag=f"acc{g}")
                for g in range(NH * NSEG)
            ]

            # ---- strip producer: hp+dst matmuls, batched exps, g casts ----
            strip_tiles = {}

            def produce(si):
                if si >= len(strips):
                    return
                c0, n = strips[si]
                st = psc.tile([128, 512], f32, name=f"hp{si}", tag="scr")
                sv = st[:, 0 : n * 130].rearrange("p (c k) -> p c k", c=n)
                strip_tiles[si] = sv
                for cc in range(n):
                    for fc in range(2):
                        nc.tensor.matmul(
                            sv[:, cc, :],
                            lhsT=hT_sb[:, fc, ts(c0 + cc, 128)],
                            rhs=wall_sb[:, fc, :],
                            start=(fc == 0),
                            stop=(fc == 1),
                        )
                # binv = e^{-0.8 dst}  [128, n, NH]
                nc.scalar.activation(
                    binv_sb[:, c0 : c0 + n, :],
                    sv[:, :, NH * O : NH * O + NH],
                    Act.Exp,
                    scale=-0.8,
                    bias=zerob[:, :],
                )
                # q = e^{dst - CB}: f32 copy for cast scales + bf16 copy into
                # g's denominator column
                nc.scalar.activation(
                    q_sb[:, c0 : c0 + n, :],
                    sv[:, :, NH * O : NH * O + NH],
                    Act.Exp,
                    scale=1.0,
                    bias=negcb[:, :],
                )
                nc.scalar.activation(
                    g_sb[:, :, c0 : c0 + n, O : O + 1].rearrange(
                        "p h c x -> p h (c x)"
                    ),
                    sv[:, :, NH * O : NH * O + NH].rearrange("p c h -> p h c"),
                    Act.Exp,
                    scale=1.0,
                    bias=negcb[:, :],
                )
                # g[:, h, c, 0:O] = q * hp
                for cc in range(n):
                    for h in range(NH):
                        nc.scalar.activation(
                            g_sb[:, h, c0 + cc, 0:O],
                            sv[:, cc, ts(h, O)],
                            Act.Copy,
                            scale=q_sb[:, c0 + cc, h : h + 1],
                        )

            LOOKAHEAD = 2
            for si in range(LOOKAHEAD):
                produce(si)

            # ---- main loop: one fused DVE op + 2 accumulating matmuls per
            # (chunk, head)
            for si, (c0, n) in enumerate(strips):
                produce(si + LOOKAHEAD)
                for cc in range(n):
                    c = c0 + cc
                    for h in range(NH):
                        r_t = sbr.tile(
                            [128, NI], bf16, name=f"r_{h}_{c}", tag=f"R{h}", bufs=4
                        )
                        nc.vector.tensor_scalar_max(
                            r_t[:, :],
                            esrc_rep[:, h, :],
                            binv_sb[:, c, h : h + 1],
                        )
                        p_t = sbr.tile(
                            [128, NI], bf16, name=f"p_{h}_{c}", tag=f"P{h}", bufs=8
                        )
                        nc.vector.tensor_tensor(
                            out=p_t[:, :],
                            in0=r_t[:, :],
                            in1=adjT_sb[:, c, :],
                            op=Alu.mult,
                        )
                        for seg in range(NSEG):
                            nc.tensor.matmul(
                                acc[h * NSEG + seg][:, :],
                                lhsT=g_sb[:, h, c, :],
                                rhs=p_t[:, ts(seg, 512)],
                                start=(c == 0),
                                stop=(c == NCJ - 1),
                            )

            # ---- epilogue: transpose [65,512] -> 4x[128,65], divide by the
            # denominator column, stage, DMA out per head
            for h in range(NH):
                for seg in range(NSEG):
                    a_ps = acc[h * NSEG + seg]
                    tr_in = sbo.tile([O + 1, 512], f32, name=f"tr_{h}_{seg}", tag="trin")
                    nc.scalar.copy(tr_in[:, :], a_ps[:, :])
                    trp = psc.tile([128, 512], f32, name=f"trp_{h}_{seg}", tag="scr")
                    trv = trp[:, 0 : 4 * (O + 1)].rearrange("p (q k) -> p q k", q=4)
                    for q in range(4):
                        nc.tensor.transpose(
                            trv[:, q, :],
                            tr_in[:, ts(q, 128)],
                            ident[0 : O + 1, 0 : O + 1],
                        )
                    rec = sbr.tile([128, 4], f32, name=f"rec_{h}_{seg}", tag="rec")
                    nc.vector.reciprocal(
                        rec[:, :], trv[:, :, O : O + 1].rearrange("p q x -> p (q x)")
                    )
                    for q in range(4):
                        nc.scalar.activation(
                            ostage[:, h, seg * 4 + q, :],
                            trv[:, q, 0:O],
                            Act.Copy,
                            scale=rec[:, q : q + 1],
                        )
                nc.sync.dma_start(
                    out_d[h, :, :], ostage[:, h, :, :].rearrange("p s o -> p (s o)")
                )

    nc.finalize()
    return nc


def _prep_inputs(h, adj, w, a_src, a_dst, bias):
    """Host-side sharding / layout prep (no reference math)."""
    h = np.asarray(h, dtype=np.float32)
    adj = np.asarray(adj)
    w = np.asarray(w, dtype=np.float32)
    a_src = np.asarray(a_src, dtype=np.float32)
    a_dst = np.asarray(a_dst, dtype=np.float32)
    bias = np.asarray(bias, dtype=np.float32)

    hT = np.ascontiguousarray(h.T)                       # [F, N]
    adjT = np.ascontiguousarray(adj.T).astype(ml_dtypes.bfloat16)  # [N, N] 0/1

    in_maps = []
    for c in range(NCORES):
        hb, ib = c % 2, c // 2
        heads = [2 * hb, 2 * hb + 1]
        i0 = NI * ib
        w2 = w[heads]                                    # [2, F, O]
        wr = np.ascontiguousarray(w2.transpose(1, 0, 2).reshape(F, NH * O))
        wTr = np.ascontiguousarray(
            np.concatenate([w2[0].T, w2[1].T], axis=1)   # [O, 2F]
        )
        avec = np.ascontiguousarray(
            np.stack(
                [a_src[heads[0], :, 0], a_dst[heads[0], :, 0],
                 a_src[heads[1], :, 0], a_dst[heads[1], :, 0]],
                axis=1,
            )
        )                                                # [O, 4]
        in_maps.append(
            {
                "hT": hT.astype(ml_dtypes.bfloat16),
                "hTi": np.ascontiguousarray(hT[:, i0 : i0 + NI]).astype(
                    ml_dtypes.bfloat16
                ),
                "adjT": np.ascontiguousarray(
                    adjT[:, i0 : i0 + NI]
                    .reshape(8, 4, 128, NI)
                    .transpose(0, 2, 1, 3)
                    .reshape(8 * 128, 4 * NI)
                ),
                "wr": wr.astype(ml_dtypes.bfloat16),
                "wTr": wTr.astype(ml_dtypes.bfloat16),
                "avec": avec.astype(ml_dtypes.bfloat16),
            }
        )
    return in_maps


def kernel(h, adj, w, a_src, a_dst, bias):
    from concourse.bass_utils import run_bass_kernel_spmd

    if "nc" not in _CACHE:
        _CACHE["nc"] = _build()
    nc = _CACHE["nc"]

    in_maps = _prep_inputs(h, adj, w, a_src, a_dst, bias)
    res = run_bass_kernel_spmd(nc, in_maps, list(range(NCORES))).results

    out = np.empty((N, NHEAD, O), dtype=np.float32)
    for c in range(NCORES):
        hb, ib = c % 2, c // 2
        arr = res[c]["out"]  # [NH, 128, NSUB*O]
        for hh in range(NH):
            blk = (
                arr[hh]
                .reshape(128, NSUB, O)
                .transpose(1, 0, 2)
                .reshape(NI, O)
            )
            out[NI * ib : NI * (ib + 1), 2 * hb + hh, :] = blk
    out += np.asarray(bias, dtype=np.float32).reshape(1, 1, O)
    return out


# revision 14
# speedup vs baseline: 1.1449x; 1.0169x over previous
"""Multi-head graph attention (GAT) Trainium2 kernel, 8-core SPMD.

Problem: h[4096,256], adj[4096,4096] bool, w[4,256,64], a_src/a_dst[4,64,1],
bias[64] -> out[4096,4,64]:
    h_prime = h @ w[k]                       per head
    s[i,j]  = src[i] + dst[j]                (rank-1!)
    scores  = leaky_relu(s, 0.2), masked by adj, softmax over j
    out     = attn @ h_prime + bias

Sharding: 8 cores = 2 head-groups x 4 row-blocks. Core c computes heads
[2*(c%2), 2*(c%2)+1] for output rows [1024*(c//2), 1024*(c//2)+1024).

Key algebra (per head, all on-device):
    exp(leaky(s)) = e^{0.2 s} * max(e^{0.8 s}, 1)
factors into row/col terms; the e^{0.2 src_i} column factor cancels in the
softmax.  With a = e^{0.8 src_i}, binv_j = e^{-0.8 dst_j},
q_j = e^{dst_j - CB}:
    max(e^{0.8(src+dst)}, 1) = e^{0.8 dst_j} * max(a_i, binv_j)
so the unnormalized weight splits as
    P[j,i] = max(a_i, binv_j) * adj[j,i]          (ONE fused DVE op:
             scalar_tensor_tensor (a max binv) mult adjT)
    g[j,o] = q_j * hp[j,o],  g[j,64] = q_j        (the j-row factor rides
             the stationary matmul operand; col 64 gives the softmax
             denominator for free)
The epilogue transposes acc[65,512] -> [128,65] and divides by the
denominator column.  hp/dst are computed in 3-chunk psum strips so the
exp/copy ACT work batches into a handful of wide ops instead of ~200
narrow ones.
"""

import sys

sys.path.insert(0, "/opt/trn_rl_repo")

import numpy as np
import ml_dtypes

N = 4096          # nodes
F = 256           # f_in
O = 64            # f_out
NHEAD = 4
NCORES = 8
NH = 2            # heads per core
NI = 1024         # output rows per core
NCJ = N // 128    # 32 j-chunks
NSEG = NI // 512  # 2 segments of 512 in the i (free) dim
NSUB = NI // 128  # 8 i-subtiles of 128
CB = 20.0         # shift inside e^{dst - CB} to keep ranges safe
SCHUNK = 3        # j-chunks per hp/dst psum strip

_CACHE = {}


def _build():
    import concourse.bass as bass
    import concourse.bacc as bacc
    import concourse.mybir as mybir
    import concourse.tile as tile
    from concourse.bass import ts

    from concourse.masks import make_identity

    f32 = mybir.dt.float32
    bf16 = mybir.dt.bfloat16
    Alu = mybir.AluOpType
    Act = mybir.ActivationFunctionType

    nc = bacc.Bacc()
    hT_d = nc.declare_dram_parameter("hT", [F, N], bf16, isOutput=False)
    hTi_d = nc.declare_dram_parameter("hTi", [F, NI], bf16, isOutput=False)
    adjT_d = nc.declare_dram_parameter("adjT", [8 * 128, 4 * NI], bf16, isOutput=False)
    wr_d = nc.declare_dram_parameter("wr", [F, NH * O], bf16, isOutput=False)
    wTr_d = nc.declare_dram_parameter("wTr", [O, NH * F], bf16, isOutput=False)
    avec_d = nc.declare_dram_parameter("avec", [O, 2 * NH], bf16, isOutput=False)
    out_d = nc.declare_dram_parameter("out", [NH, 128, NSUB * O], f32, isOutput=True)

    # strips of SCHUNK j-chunks; each strip is one [128, n*130] psum region
    strips = []
    c0 = 0
    while c0 < NCJ:
        n = min(SCHUNK, NCJ - c0)
        strips.append((c0, n))
        c0 += n

    with tile.TileContext(nc) as tc:
        with (
            tc.tile_pool(name="sb", bufs=1) as sb,
            tc.tile_pool(name="sbr", bufs=2) as sbr,
            tc.tile_pool(name="sbo", bufs=3) as sbo,
            tc.tile_pool(name="psc", bufs=3, space="PSUM") as psc,
            tc.tile_pool(name="pacc", bufs=1, space="PSUM") as pacc,
        ):
            # ---- static SBUF tensors ----
            hT_sb = sb.tile([128, 2, N], bf16, name="hT_sb")
            hTi_sb = sb.tile([128, 2, NI], bf16, name="hTi_sb")
            adjT_sb = sb.tile([128, NCJ, NI], bf16, name="adjT_sb")
            wTr_sb = sb.tile([O, NH, F], bf16, name="wTr_sb")
            avec_sb = sb.tile([O, 2 * NH], bf16, name="avec_sb")
            wall_sb = sb.tile([128, 2, NH * O + NH], bf16, name="wall_sb")
            vsrc_sb = sb.tile([128, 2, NH], bf16, name="vsrc_sb")
            ones_row = sb.tile([1, 128], bf16, name="ones_row")
            esrc_row = sb.tile([1, NH, NI], bf16, name="esrc_row")
            esrc_rep = sb.tile([128, NH, NI], bf16, name="esrc_rep")
            g_sb = sb.tile([128, NH, NCJ, O + 1], bf16, name="g_sb")
            binv_sb = sb.tile([128, NCJ, NH], f32, name="binv_sb")
            q_sb = sb.tile([128, NCJ, NH], f32, name="q_sb")
            ostage = sb.tile([128, NH, NSUB, O], f32, name="ostage")
            negcb = sb.tile([128, 1], f32, name="negcb")
            nc.vector.memset(negcb[:, :], -CB)
            zerob = sb.tile([128, 1], f32, name="zerob")
            nc.vector.memset(zerob[:, :], 0.0)
            ident = sb.tile([128, 128], f32, name="ident")
            make_identity(nc, ident[:, :])

            # ---- DMA in ----  (small control tensors FIRST so the prologue
            # matmul chain can start while the bulk hT/adjT loads stream in)
            nc.sync.dma_start(
                hTi_sb, hTi_d[:, :].rearrange("(fc p) i -> p fc i", p=128)
            )
            wTr_r = wTr_d[:, :].rearrange("o (h f) -> o h f", h=NH)
            for h in range(NH):
                nc.sync.dma_start(wTr_sb[:, h, :], wTr_r[:, h, :])
            nc.sync.dma_start(avec_sb, avec_d[:, :])
            nc.sync.dma_start(
                wall_sb[:, :, 0 : NH * O],
                wr_d[:, :].rearrange("(fc p) m -> p fc m", p=128),
            )
            # hT first half before adjT: the hp-strip chain (strip matmuls ->
            # binv exp -> first DVE op) is the startup critical path
            adjT_r = adjT_d[:, :].rearrange("(g p) x -> g p x", p=128)
            hT_r = hT_d[:, :].rearrange("(fc p) j -> p fc j", p=128)
            nc.sync.dma_start(hT_sb[:, :, 0:512], hT_r[:, :, 0:512])
            nc.sync.dma_start(
                adjT_sb[:, 0:4, :].rearrange("p c i -> p (c i)"), adjT_r[0]
            )
            nc.sync.dma_start(hT_sb[:, :, 512:N], hT_r[:, :, 512:N])
            for g in range(1, 7, 2):
                nc.sync.dma_start(
                    adjT_sb[:, 4 * g : 4 * g + 8, :].rearrange(
                        "p (g c) i -> p g (c i)", g=2
                    ),
                    adjT_r[g : g + 2].rearrange("g p x -> p g x"),
                )
            nc.sync.dma_start(
                adjT_sb[:, 28:32, :].rearrange("p c i -> p (c i)"), adjT_r[7]
            )

            nc.vector.memset(ones_row[:, :], 1.0)

            # ---- v vectors: v[f] = sum_o wT[o,f] * a[o]  (src, dst per head)
            v_ps = psc.tile([128, 512], f32, name="v_ps", tag="scr")
            for h in range(NH):
                for fc in range(2):
                    nc.tensor.matmul(
                        v_ps[:, 4 * fc + 2 * h : 4 * fc + 2 * h + 2],
                        lhsT=wTr_sb[:, h, ts(fc, 128)],
                        rhs=avec_sb[:, 2 * h : 2 * h + 2],
                        start=True,
                        stop=True,
                    )
            for fc in range(2):
                for h in range(NH):
                    nc.scalar.copy(
                        vsrc_sb[:, fc, h : h + 1], v_ps[:, 4 * fc + 2 * h : 4 * fc + 2 * h + 1]
                    )
                    nc.scalar.copy(
                        wall_sb[:, fc, NH * O + h : NH * O + h + 1],
                        v_ps[:, 4 * fc + 2 * h + 1 : 4 * fc + 2 * h + 2],
                    )

            # ---- src rows for this core's i-block (both heads), then exp
            for h in range(NH):
                for seg in range(NSEG):
                    sr_ps = psc.tile([128, 512], f32, name=f"sr_ps_{h}_{seg}", tag="scr")
                    for fc in range(2):
                        nc.tensor.matmul(
                            sr_ps[0:1, :],
                            lhsT=vsrc_sb[:, fc, h : h + 1],
                            rhs=hTi_sb[:, fc, ts(seg, 512)],
                            start=(fc == 0),
                            stop=(fc == 1),
                        )
                    nc.scalar.activation(
                        esrc_row[0:1, h, ts(seg, 512)],
                        sr_ps[0:1, :],
                        Act.Exp,
                        scale=0.8,
                        bias=zerob[0:1, :],
                    )

            # ---- replicate e^{0.8 src} across partitions (K=1 ones matmul)
            for h in range(NH):
                for seg in range(NSEG):
                    rep_ps = psc.tile([128, 512], f32, name=f"rep_ps_{h}_{seg}", tag="scr")
                    nc.tensor.matmul(
                        rep_ps[:, :],
                        lhsT=ones_row[:, :],
                        rhs=esrc_row[0:1, h, ts(seg, 512)],
                        start=True,
                        stop=True,
                    )
                    nc.scalar.copy(esrc_rep[:, h, ts(seg, 512)], rep_ps[:, :])

            # ---- bmm accumulators: psum [65, 512] per (head, i-segment)
            acc = [
                pacc.tile([O + 1, 512], f32, name=f"acc{g}", tag=f"acc{g}")
                for g in range(NH * NSEG)
            ]

            # ---- strip producer: hp+dst matmuls, batched exps, g casts ----
            strip_tiles = {}

            def produce(si):
                if si >= len(strips):
                    return
                c0, n = strips[si]
                st = psc.tile([128, 512], f32, name=f"hp{si}", tag="scr")
                sv = st[:, 0 : n * 130].rearrange("p (c k) -> p c k", c=n)
                strip_tiles[si] = sv
                for cc in range(n):
                    for fc in range(2):
                        nc.tensor.matmul(
                            sv[:, cc, :],
                            lhsT=hT_sb[:, fc, ts(c0 + cc, 128)],
                            rhs=wall_sb[:, fc, :],
                            start=(fc == 0),
                            stop=(fc == 1),
                        )
                # binv = e^{-0.8 dst}  [128, n, NH]
                nc.scalar.activation(
                    binv_sb[:, c0 : c0 + n, :],
                    sv[:, :, NH * O : NH * O + NH],
                    Act.Exp,
                    scale=-0.8,
                    bias=zerob[:, :],
                )
                # q = e^{dst - CB}: f32 copy for cast scales + bf16 copy into
                # g's denominator column
                nc.scalar.activation(
                    q_sb[:, c0 : c0 + n, :],
                    sv[:, :, NH * O : NH * O + NH],
                    Act.Exp,
                    scale=1.0,
                    bias=negcb[:, :],
                )
                nc.scalar.activation(
                    g_sb[:, :, c0 : c0 + n, O : O + 1].rearrange(
                        "p h c x -> p h (c x)"
                    ),
                    sv[:, :, NH * O : NH * O + NH].rearrange("p c h -> p h c"),
                    Act.Exp,
                    scale=1.0,
                    bias=negcb[:, :],
                )
                # g[:, h, c, 0:O] = q * hp
                for cc in range(n):
                    for h in range(NH):
                        nc.scalar.activation(
                            g_sb[:, h, c0 + cc, 0:O],
                            sv[:, cc, ts(h, O)],
                            Act.Copy,
                            scale=q_sb[:, c0 + cc, h : h + 1],
                        )

            LOOKAHEAD = 2
            for si in range(LOOKAHEAD):
                produce(si)

            # ---- main loop, split by head: pass h=0 drives the strip
            # producer; pass h=1 runs from SBUF-resident state while head 0's
            # epilogue overlaps.  Per (strip, head): n tensor_scalar_max ops,
            # ONE wide tensor_tensor mask-multiply over the whole strip, and
            # 2n accumulating matmuls.
            def epilogue(h):
                for seg in range(NSEG):
                    a_ps = acc[h * NSEG + seg]
                    tr_in = sbo.tile([O + 1, 512], f32, name=f"tr_{h}_{seg}", tag="trin")
                    nc.scalar.copy(tr_in[:, :], a_ps[:, :])
                    trp = psc.tile([128, 512], f32, name=f"trp_{h}_{seg}", tag="scr")
                    trv = trp[:, 0 : 4 * (O + 1)].rearrange("p (q k) -> p q k", q=4)
                    for q in range(4):
                        nc.tensor.transpose(
                            trv[:, q, :],
                            tr_in[:, ts(q, 128)],
                            ident[0 : O + 1, 0 : O + 1],
                        )
                    rec = sbr.tile([128, 4], f32, name=f"rec_{h}_{seg}", tag="rec")
                    nc.vector.reciprocal(
                        rec[:, :], trv[:, :, O : O + 1].rearrange("p q x -> p (q x)")
                    )
                    for q in range(4):
                        nc.scalar.activation(
                            ostage[:, h, seg * 4 + q, :],
                            trv[:, q, 0:O],
                            Act.Copy,
                            scale=rec[:, q : q + 1],
                        )
                nc.sync.dma_start(
                    out_d[h, :, :], ostage[:, h, :, :].rearrange("p s o -> p (s o)")
                )

            for h in range(NH):
                for si, (c0, n) in enumerate(strips):
                    if h == 0:
                        produce(si + LOOKAHEAD)
                    r3 = sbr.tile(
                        [128, SCHUNK, NI], bf16, name=f"r_{h}_{si}", tag="R", bufs=3
                    )
                    for cc in range(n):
                        nc.vector.tensor_scalar_max(
                            r3[:, cc, :],
                            esrc_rep[:, h, :],
                            binv_sb[:, c0 + cc, h : h + 1],
                        )
                    p3 = sbr.tile(
                        [128, SCHUNK, NI], bf16, name=f"p_{h}_{si}", tag="P", bufs=3
                    )
                    nc.vector.tensor_tensor(
                        out=p3[:, 0:n, :].rearrange("p c i -> p (c i)"),
                        in0=r3[:, 0:n, :].rearrange("p c i -> p (c i)"),
                        in1=adjT_sb[:, c0 : c0 + n, :].rearrange("p c i -> p (c i)"),
                        op=Alu.mult,
                    )
                    for cc in range(n):
                        c = c0 + cc
                        for seg in range(NSEG):
                            nc.tensor.matmul(
                                acc[h * NSEG + seg][:, :],
                                lhsT=g_sb[:, h, c, :],
                                rhs=p3[:, cc, ts(seg, 512)],
                                start=(c == 0),
                                stop=(c == NCJ - 1),
                            )
                epilogue(h)

    nc.finalize()
    return nc


def _prep_inputs(h, adj, w, a_src, a_dst, bias):
    """Host-side sharding / layout prep (no reference math)."""
    h = np.asarray(h, dtype=np.float32)
    adj = np.asarray(adj)
    w = np.asarray(w, dtype=np.float32)
    a_src = np.asarray(a_src, dtype=np.float32)
    a_dst = np.asarray(a_dst, dtype=np.float32)
    bias = np.asarray(bias, dtype=np.float32)

    hT = np.ascontiguousarray(h.T)                       # [F, N]
    adjT = np.ascontiguousarray(adj.T).astype(ml_dtypes.bfloat16)  # [N, N] 0/1

    in_maps = []
    for c in range(NCORES):
        hb, ib = c % 2, c // 2
        heads = [2 * hb, 2 * hb + 1]
        i0 = NI * ib
        w2 = w[heads]                                    # [2, F, O]
        wr = np.ascontiguousarray(w2.transpose(1, 0, 2).reshape(F, NH * O))
        wTr = np.ascontiguousarray(
            np.concatenate([w2[0].T, w2[1].T], axis=1)   # [O, 2F]
        )
        avec = np.ascontiguousarray(
            np.stack(
                [a_src[heads[0], :, 0], a_dst[heads[0], :, 0],
                 a_src[heads[1], :, 0], a_dst[heads[1], :, 0]],
                axis=1,
            )
        )                                                # [O, 4]
        in_maps.append(
            {
                "hT": hT.astype(ml_dtypes.bfloat16),
                "hTi": np.ascontiguousarray(hT[:, i0 : i0 + NI]).astype(
                    ml_dtypes.bfloat16
                ),
                "adjT": np.ascontiguousarray(
                    adjT[:, i0 : i0 + NI]
                    .reshape(8, 4, 128, NI)
                    .transpose(0, 2, 1, 3)
                    .reshape(8 * 128, 4 * NI)
                ),
                "wr": wr.astype(ml_dtypes.bfloat16),
                "wTr": wTr.astype(ml_dtypes.bfloat16),
                "avec": avec.astype(ml_dtypes.bfloat16),
            }
        )
    return in_maps


def kernel(h, adj, w, a_src, a_dst, bias):
    from concourse.bass_utils import run_bass_kernel_spmd

    if "nc" not in _CACHE:
        _CACHE["nc"] = _build()
    nc = _CACHE["nc"]

    in_maps = _prep_inputs(h, adj, w, a_src, a_dst, bias)
    res = run_bass_kernel_spmd(nc, in_maps, list(range(NCORES))).results

    out = np.empty((N, NHEAD, O), dtype=np.float32)
    for c in range(NCORES):
        hb, ib = c % 2, c // 2
        arr = res[c]["out"]  # [NH, 128, NSUB*O]
        for hh in range(NH):
            blk = (
                arr[hh]
                .reshape(128, NSUB, O)
                .transpose(1, 0, 2)
                .reshape(NI, O)
            )
            out[NI * ib : NI * (ib + 1), 2 * hb + hh, :] = blk
    out += np.asarray(bias, dtype=np.float32).reshape(1, 1, O)
    return out


# revision 21
# speedup vs baseline: 1.1515x; 1.0058x over previous
"""Multi-head graph attention (GAT) Trainium2 kernel, 8-core SPMD.

Problem: h[4096,256], adj[4096,4096] bool, w[4,256,64], a_src/a_dst[4,64,1],
bias[64] -> out[4096,4,64]:
    h_prime = h @ w[k]                       per head
    s[i,j]  = src[i] + dst[j]                (rank-1!)
    scores  = leaky_relu(s, 0.2), masked by adj, softmax over j
    out     = attn @ h_prime + bias

Sharding: 8 cores = 2 head-groups x 4 row-blocks. Core c computes heads
[2*(c%2), 2*(c%2)+1] for output rows [1024*(c//2), 1024*(c//2)+1024).

Key algebra (per head, all on-device):
    exp(leaky(s)) = e^{0.2 s} * max(e^{0.8 s}, 1)
factors into row/col terms; the e^{0.2 src_i} column factor cancels in the
softmax.  With a = e^{0.8 src_i}, binv_j = e^{-0.8 dst_j},
q_j = e^{dst_j - CB}:
    max(e^{0.8(src+dst)}, 1) = e^{0.8 dst_j} * max(a_i, binv_j)
so the unnormalized weight splits as
    P[j,i] = max(a_i, binv_j) * adj[j,i]          (ONE fused DVE op:
             scalar_tensor_tensor (a max binv) mult adjT)
    g[j,o] = q_j * hp[j,o],  g[j,64] = q_j        (the j-row factor rides
             the stationary matmul operand; col 64 gives the softmax
             denominator for free)
The epilogue transposes acc[65,512] -> [128,65] and divides by the
denominator column.  hp/dst are computed in 3-chunk psum strips so the
exp/copy ACT work batches into a handful of wide ops instead of ~200
narrow ones.
"""

import sys

sys.path.insert(0, "/opt/trn_rl_repo")

import numpy as np
import ml_dtypes

N = 4096          # nodes
F = 256           # f_in
O = 64            # f_out
NHEAD = 4
NCORES = 8
NH = 2            # heads per core
NI = 1024         # output rows per core
NCJ = N // 128    # 32 j-chunks
NSEG = NI // 512  # 2 segments of 512 in the i (free) dim
NSUB = NI // 128  # 8 i-subtiles of 128
CB = 20.0         # shift inside e^{dst - CB} to keep ranges safe
SCHUNK = 3        # j-chunks per hp/dst psum strip

_CACHE = {}


def _build():
    import concourse.bass as bass
    import concourse.bacc as bacc
    import concourse.mybir as mybir
    import concourse.tile as tile
    from concourse.bass import ts

    from concourse.masks import make_identity

    f32 = mybir.dt.float32
    bf16 = mybir.dt.bfloat16
    Alu = mybir.AluOpType
    Act = mybir.ActivationFunctionType

    nc = bacc.Bacc()
    hT_d = nc.declare_dram_parameter("hT", [F, N], bf16, isOutput=False)
    hTi_d = nc.declare_dram_parameter("hTi", [F, NI], bf16, isOutput=False)
    adjT_d = nc.declare_dram_parameter("adjT", [8 * 128, 4 * NI], bf16, isOutput=False)
    # wall = [w (2 heads x 64 cols) | vdst (2 cols)], vsrc separate; the
    # v-vectors (w @ a) are host-folded weight prep
    wall_d = nc.declare_dram_parameter("wall", [F, NH * O + NH], bf16, isOutput=False)
    vsrc_d = nc.declare_dram_parameter("vsrc", [F, NH], bf16, isOutput=False)
    out_d = nc.declare_dram_parameter("out", [NH, 128, NSUB * O], f32, isOutput=True)

    # strips of SCHUNK j-chunks; each strip is one [128, n*130] psum region
    strips = []
    c0 = 0
    while c0 < NCJ:
        n = min(SCHUNK, NCJ - c0)
        strips.append((c0, n))
        c0 += n

    with tile.TileContext(nc) as tc:
        with (
            tc.tile_pool(name="sb", bufs=1) as sb,
            tc.tile_pool(name="sbr", bufs=2) as sbr,
            tc.tile_pool(name="sbo", bufs=3) as sbo,
            tc.tile_pool(name="psc", bufs=3, space="PSUM") as psc,
            tc.tile_pool(name="pacc", bufs=1, space="PSUM") as pacc,
        ):
            # ---- static SBUF tensors ----
            hT_sb = sb.tile([128, 2, N], bf16, name="hT_sb")
            hTi_sb = sb.tile([128, 2, NI], bf16, name="hTi_sb")
            adjT_sb = sb.tile([128, NCJ, NI], bf16, name="adjT_sb")
            wall_sb = sb.tile([128, 2, NH * O + NH], bf16, name="wall_sb")
            vsrc_sb = sb.tile([128, 2, NH], bf16, name="vsrc_sb")
            ones_row = sb.tile([1, 128], bf16, name="ones_row")
            esrc_row = sb.tile([1, NH, NI], bf16, name="esrc_row")
            esrc_rep = sb.tile([128, NH, NI], bf16, name="esrc_rep")
            g_sb = sb.tile([128, NH, NCJ, O + 1], bf16, name="g_sb")
            binv_sb = sb.tile([128, NCJ, NH], f32, name="binv_sb")
            q_sb = sb.tile([128, NCJ, NH], f32, name="q_sb")
            ostage = sb.tile([128, NH, NSUB, O], f32, name="ostage")
            negcb = sb.tile([128, 1], f32, name="negcb")
            nc.vector.memset(negcb[:, :], -CB)
            zerob = sb.tile([128, 1], f32, name="zerob")
            nc.vector.memset(zerob[:, :], 0.0)
            ident = sb.tile([128, 128], f32, name="ident")
            make_identity(nc, ident[:, :])

            # ---- DMA in ----  (small control tensors FIRST so the prologue
            # matmul chain can start while the bulk hT/adjT loads stream in)
            nc.sync.dma_start(
                hTi_sb, hTi_d[:, :].rearrange("(fc p) i -> p fc i", p=128)
            )
            nc.sync.dma_start(
                vsrc_sb, vsrc_d[:, :].rearrange("(fc p) m -> p fc m", p=128)
            )
            nc.sync.dma_start(
                wall_sb, wall_d[:, :].rearrange("(fc p) m -> p fc m", p=128)
            )
            # hT first half before adjT: the hp-strip chain (strip matmuls ->
            # binv exp -> first DVE op) is the startup critical path
            adjT_r = adjT_d[:, :].rearrange("(g p) x -> g p x", p=128)
            hT_r = hT_d[:, :].rearrange("(fc p) j -> p fc j", p=128)
            nc.sync.dma_start(hT_sb[:, :, 0:512], hT_r[:, :, 0:512])
            nc.sync.dma_start(
                adjT_sb[:, 0:4, :].rearrange("p c i -> p (c i)"), adjT_r[0]
            )
            nc.sync.dma_start(hT_sb[:, :, 512:N], hT_r[:, :, 512:N])
            for g in range(1, 7, 2):
                nc.sync.dma_start(
                    adjT_sb[:, 4 * g : 4 * g + 8, :].rearrange(
                        "p (g c) i -> p g (c i)", g=2
                    ),
                    adjT_r[g : g + 2].rearrange("g p x -> p g x"),
                )
            nc.sync.dma_start(
                adjT_sb[:, 28:32, :].rearrange("p c i -> p (c i)"), adjT_r[7]
            )

            nc.vector.memset(ones_row[:, :], 1.0)

            # ---- src rows for this core's i-block (both heads), then exp
            for h in range(NH):
                for seg in range(NSEG):
                    sr_ps = psc.tile([128, 512], f32, name=f"sr_ps_{h}_{seg}", tag="scr")
                    for fc in range(2):
                        nc.tensor.matmul(
                            sr_ps[0:1, :],
                            lhsT=vsrc_sb[:, fc, h : h + 1],
                            rhs=hTi_sb[:, fc, ts(seg, 512)],
                            start=(fc == 0),
                            stop=(fc == 1),
                        )
                    nc.scalar.activation(
                        esrc_row[0:1, h, ts(seg, 512)],
                        sr_ps[0:1, :],
                        Act.Exp,
                        scale=0.8,
                        bias=zerob[0:1, :],
                    )

            # ---- replicate e^{0.8 src} across partitions (K=1 ones matmul)
            for h in range(NH):
                for seg in range(NSEG):
                    rep_ps = psc.tile([128, 512], f32, name=f"rep_ps_{h}_{seg}", tag="scr")
                    nc.tensor.matmul(
                        rep_ps[:, :],
                        lhsT=ones_row[:, :],
                        rhs=esrc_row[0:1, h, ts(seg, 512)],
                        start=True,
                        stop=True,
                    )
                    nc.vector.tensor_copy(esrc_rep[:, h, ts(seg, 512)], rep_ps[:, :])

            # ---- bmm accumulators: psum [65, 512] per (head, i-segment)
            acc = [
                pacc.tile([O + 1, 512], f32, name=f"acc{g}", tag=f"acc{g}")
                for g in range(NH * NSEG)
            ]

            # ---- strip producer: hp+dst matmuls, batched exps, g casts ----
            strip_tiles = {}

            def produce(si):
                if si >= len(strips):
                    return
                c0, n = strips[si]
                st = psc.tile([128, 512], f32, name=f"hp{si}", tag="scr")
                sv = st[:, 0 : n * 130].rearrange("p (c k) -> p c k", c=n)
                strip_tiles[si] = sv
                for cc in range(n):
                    for fc in range(2):
                        nc.tensor.matmul(
                            sv[:, cc, :],
                            lhsT=hT_sb[:, fc, ts(c0 + cc, 128)],
                            rhs=wall_sb[:, fc, :],
                            start=(fc == 0),
                            stop=(fc == 1),
                        )
                # binv = e^{-0.8 dst}  [128, n, NH]
                nc.scalar.activation(
                    binv_sb[:, c0 : c0 + n, :],
                    sv[:, :, NH * O : NH * O + NH],
                    Act.Exp,
                    scale=-0.8,
                    bias=zerob[:, :],
                )
                # q = e^{dst - CB}: f32 copy for cast scales + bf16 copy into
                # g's denominator column
                nc.scalar.activation(
                    q_sb[:, c0 : c0 + n, :],
                    sv[:, :, NH * O : NH * O + NH],
                    Act.Exp,
                    scale=1.0,
                    bias=negcb[:, :],
                )
                nc.scalar.activation(
                    g_sb[:, :, c0 : c0 + n, O : O + 1].rearrange(
                        "p h c x -> p h (c x)"
                    ),
                    sv[:, :, NH * O : NH * O + NH].rearrange("p c h -> p h c"),
                    Act.Exp,
                    scale=1.0,
                    bias=negcb[:, :],
                )
                # g[:, h, c, 0:O] = q * hp
                for cc in range(n):
                    for h in range(NH):
                        nc.scalar.activation(
                            g_sb[:, h, c0 + cc, 0:O],
                            sv[:, cc, ts(h, O)],
                            Act.Copy,
                            scale=q_sb[:, c0 + cc, h : h + 1],
                        )

            LOOKAHEAD = 2
            for si in range(LOOKAHEAD):
                produce(si)

            # ---- main loop, split by head: pass h=0 drives the strip
            # producer; pass h=1 runs from SBUF-resident state while head 0's
            # epilogue overlaps.  Per (strip, head): n tensor_scalar_max ops,
            # ONE wide tensor_tensor mask-multiply over the whole strip, and
            # 2n accumulating matmuls.
            def epilogue(h):
                for seg in range(NSEG):
                    a_ps = acc[h * NSEG + seg]
                    tr_in = sbo.tile([O + 1, 512], f32, name=f"tr_{h}_{seg}", tag="trin")
                    nc.scalar.copy(tr_in[:, :], a_ps[:, :])
                    trp = psc.tile([128, 512], f32, name=f"trp_{h}_{seg}", tag="scr")
                    trv = trp[:, 0 : 4 * (O + 1)].rearrange("p (q k) -> p q k", q=4)
                    for q in range(4):
                        nc.tensor.transpose(
                            trv[:, q, :],
                            tr_in[:, ts(q, 128)],
                            ident[0 : O + 1, 0 : O + 1],
                        )
                    rec = sbr.tile([128, 4], f32, name=f"rec_{h}_{seg}", tag="rec")
                    nc.vector.reciprocal(
                        rec[:, :], trv[:, :, O : O + 1].rearrange("p q x -> p (q x)")
                    )
                    for q in range(4):
                        nc.scalar.activation(
                            ostage[:, h, seg * 4 + q, :],
                            trv[:, q, 0:O],
                            Act.Copy,
                            scale=rec[:, q : q + 1],
                        )
                    nc.sync.dma_start(
                        out_d[h, :, seg * 4 * O : (seg + 1) * 4 * O],
                        ostage[:, h, seg * 4 : (seg + 1) * 4, :].rearrange(
                            "p s o -> p (s o)"
                        ),
                    )

            for h in range(NH):
                for si, (c0, n) in enumerate(strips):
                    if h == 0:
                        produce(si + LOOKAHEAD)
                    r3 = sbr.tile(
                        [128, SCHUNK, NI], bf16, name=f"r_{h}_{si}", tag="R", bufs=3
                    )
                    for cc in range(n):
                        nc.vector.tensor_scalar_max(
                            r3[:, cc, :],
                            esrc_rep[:, h, :],
                            binv_sb[:, c0 + cc, h : h + 1],
                        )
                    p3 = sbr.tile(
                        [128, SCHUNK, NI], bf16, name=f"p_{h}_{si}", tag="P", bufs=3
                    )
                    nc.vector.tensor_tensor(
                        out=p3[:, 0:n, :].rearrange("p c i -> p (c i)"),
                        in0=r3[:, 0:n, :].rearrange("p c i -> p (c i)"),
                        in1=adjT_sb[:, c0 : c0 + n, :].rearrange("p c i -> p (c i)"),
                        op=Alu.mult,
                    )
                    for cc in range(n):
                        c = c0 + cc
                        for seg in range(NSEG):
                            nc.tensor.matmul(
                                acc[h * NSEG + seg][:, :],
                                lhsT=g_sb[:, h, c, :],
                                rhs=p3[:, cc, ts(seg, 512)],
                                start=(c == 0),
                                stop=(c == NCJ - 1),
                            )
                epilogue(h)

    nc.finalize()
    return nc


def _prep_inputs(h, adj, w, a_src, a_dst, bias):
    """Host-side sharding / layout prep (no reference math)."""
    h = np.asarray(h, dtype=np.float32)
    adj = np.asarray(adj)
    w = np.asarray(w, dtype=np.float32)
    a_src = np.asarray(a_src, dtype=np.float32)
    a_dst = np.asarray(a_dst, dtype=np.float32)
    bias = np.asarray(bias, dtype=np.float32)

    hT = np.ascontiguousarray(h.T)                       # [F, N]
    adjT = np.ascontiguousarray(adj.T).astype(ml_dtypes.bfloat16)  # [N, N] 0/1

    in_maps = []
    for c in range(NCORES):
        hb, ib = c % 2, c // 2
        heads = [2 * hb, 2 * hb + 1]
        i0 = NI * ib
        w2 = w[heads]                                    # [2, F, O]
        wr = w2.transpose(1, 0, 2).reshape(F, NH * O)    # [F, 128]
        # host-folded weight prep: v = w @ a  (parameter-only contraction)
        vdst = np.stack(
            [w2[k] @ a_dst[heads[k], :, 0] for k in range(NH)], axis=1
        )                                                # [F, 2]
        vsrc = np.stack(
            [w2[k] @ a_src[heads[k], :, 0] for k in range(NH)], axis=1
        )                                                # [F, 2]
        wall = np.ascontiguousarray(np.concatenate([wr, vdst], axis=1))
        in_maps.append(
            {
                "hT": hT.astype(ml_dtypes.bfloat16),
                "hTi": np.ascontiguousarray(hT[:, i0 : i0 + NI]).astype(
                    ml_dtypes.bfloat16
                ),
                "adjT": np.ascontiguousarray(
                    adjT[:, i0 : i0 + NI]
                    .reshape(8, 4, 128, NI)
                    .transpose(0, 2, 1, 3)
                    .reshape(8 * 128, 4 * NI)
                ),
                "wall": wall.astype(ml_dtypes.bfloat16),
                "vsrc": np.ascontiguousarray(vsrc).astype(ml_dtypes.bfloat16),
            }
        )
    return in_maps


def kernel(h, adj, w, a_src, a_dst, bias):
    from concourse.bass_utils import run_bass_kernel_spmd

    if "nc" not in _CACHE:
        _CACHE["nc"] = _build()
    nc = _CACHE["nc"]

    in_maps = _prep_inputs(h, adj, w, a_src, a_dst, bias)
    res = run_bass_kernel_spmd(nc, in_maps, list(range(NCORES))).results

    out = np.empty((N, NHEAD, O), dtype=np.float32)
    for c in range(NCORES):
        hb, ib = c % 2, c // 2
        arr = res[c]["out"]  # [NH, 128, NSUB*O]
        for hh in range(NH):
            blk = (
                arr[hh]
                .reshape(128, NSUB, O)
                .transpose(1, 0, 2)
                .reshape(NI, O)
            )
            out[NI * ib : NI * (ib + 1), 2 * hb + hh, :] = blk
    out += np.asarray(bias, dtype=np.float32).reshape(1, 1, O)
    return out
